# revision 26
# baseline (speedup 1.0000x reference)
"""Trainium2 Bass kernel for DotAttention (nn_DotAttention_67963562492218).

Reference computation (per batch b):
    h_in  = relu(inputs @ W_in.T)            [Li, H]
    h_mem = relu(memory @ W_mem.T)           [Lm, H]
    S     = h_in @ h_mem.T / sqrt(H)         [Li, Lm]
    P     = softmax(where(mask, S, -inf))    [Li, Lm]
    att   = P @ memory                       [Li, D]
    res   = [inputs | att]                   [Li, 2D]
    out   = res * sigmoid(res @ W_res.T)     [Li, 2D]

Device strategy (8 cores, data-parallel over batch, 2 batch items/core):
  Everything on device lives in transposed ("feature-major") layout so that
  every matmul contracts over the partition axis with no on-device
  transposes.  Host pre-transposes inputs/memory/weights (free), and the
  output comes back transposed [2D, Li] and is un-transposed on host.

  - h_inT [h, i]  = W_inT.T @ inputsT     (lhsT = W_inT tile, rhs = inputsT)
  - h_memT[h, m]  = W_memT.T @ memoryT
  - S^T   [m, i]  = h_memT.T @ h_inT      -> exp(S/sqrt(H) + mask_bias[m])
      (softmax needs no max subtraction: scores are ~N(3.6, 0.5); masked
       entries get bias -1e4 so exp underflows to exactly 0)
  - attT  [d, i]  = mem_nat.T @ E         (mem natural tile is the lhsT!)
    denom [1, i]  = ones.T   @ E          (softmax denominator via matmul)
  - attT /= denom (broadcast via SBUF->SBUF DMA of the reciprocal row)
  - gateT [s, i]  = W_resT.T @ resT, resT = [inputsT ; attT] on partitions
  - outT = resT * (0.5 + 0.5*tanh(gateT/2))   (sigmoid via tanh: keeps the
    ACT engine on the exp_and_others table set the whole kernel)

All matmuls run as float32r (fp32 operands truncated to fp22 in the PE)
which streams at 1 cycle/row for free dim >= 256 -- bf16-class throughput
at ~2^-14 relative precision.
"""

import math
import numpy as np
from contextlib import ExitStack

import bass_rust
import concourse.bass as bass
import concourse.tile as tile
from concourse import bacc, mybir
from concourse.bass_utils import run_bass_kernel_spmd

F32 = mybir.dt.float32
F32R = mybir.dt.float32r
AF = mybir.ActivationFunctionType
ALU = mybir.AluOpType

N_CORES = 8
NEG_BIAS = -10000.0

# Full problem dims
FULL_B, FULL_L, FULL_D, FULL_H = 16, 2048, 512, 512


def r32(ap):
    return ap.bitcast(F32R)


def _build_program(NB, L, D, H, IBLK=512):
    """Build + compile the per-core Bass program.

    NB: batches per core; L: sequence length (Li == Lm); D: feature dim
    (Din == Dmem); H: hidden dim; R = 2*D (residual width).
    """
    R = 2 * D
    nd = D // 128   # d-tiles (contraction tiles for h_{in,mem}; partition tiles of attT)
    nh = H // 128   # h-tiles
    nm = L // 128   # m-tiles
    ns = R // 128   # s-tiles (= r-tiles)
    nib = L // IBLK  # i-blocks
    scale = 1.0 / math.sqrt(H)

    nc = bacc.Bacc("TRN2", target_bir_lowering=False)

    inT_d = nc.declare_dram_parameter("inT", [NB, D, L], F32, isOutput=False)
    memT_d = nc.declare_dram_parameter("memT", [NB, D, L], F32, isOutput=False)
    mem_d = nc.declare_dram_parameter("mem", [NB, L, D], F32, isOutput=False)
    winT_d = nc.declare_dram_parameter("winT", [D, H], F32, isOutput=False)
    wmemT_d = nc.declare_dram_parameter("wmemT", [D, H], F32, isOutput=False)
    wresT_d = nc.declare_dram_parameter("wresT", [R, R], F32, isOutput=False)
    mbias_d = nc.declare_dram_parameter("mbias", [NB, 128, nm], F32, isOutput=False)
    ones_d = nc.declare_dram_parameter("ones", [128, 1], F32, isOutput=False)
    outT_d = nc.declare_dram_parameter("outT", [NB, R, L], F32, isOutput=True)

    with tile.TileContext(nc) as tc:
        with ExitStack() as ctx:
            p_const = ctx.enter_context(tc.tile_pool(name="const", bufs=1))
            p_batch = ctx.enter_context(tc.tile_pool(name="batch", bufs=1))
            p_memT = ctx.enter_context(tc.tile_pool(name="memT", bufs=2))
            p_inT = ctx.enter_context(tc.tile_pool(name="inT", bufs=2))
            p_hin = ctx.enter_context(tc.tile_pool(name="hin", bufs=1))
            p_E = ctx.enter_context(tc.tile_pool(name="E", bufs=3))
            p_attn = ctx.enter_context(tc.tile_pool(name="attn", bufs=1))
            p_sm = ctx.enter_context(tc.tile_pool(name="sm", bufs=2))
            p_out = ctx.enter_context(tc.tile_pool(name="out", bufs=3))
            p_mm = ctx.enter_context(tc.tile_pool(name="mm", bufs=3, space="PSUM"))
            p_att = ctx.enter_context(tc.tile_pool(name="att", bufs=1, space="PSUM"))

            # ---- constants ----
            # Emission order here is DMA-queue order: load only what the PE
            # needs first (W_memT + first memoryT chunk), defer the rest so
            # the PE isn't stalled ~40us behind a serial DMA prologue.
            wres_sb = p_const.tile([128, ns, R], F32R)
            win_sb = [p_const.tile([128, H], F32R, name=f"win{dt}")
                      for dt in range(nd)]
            wmem_sb = [p_const.tile([128, H], F32R, name=f"wmem{dt}")
                       for dt in range(nd)]
            for dt in range(nd):
                # split the very first loads across HWDGE (sync) and SWDGE
                # (gpsimd) so they don't serialize on one DMA queue
                eng = nc.sync if dt % 2 == 0 else nc.gpsimd
                eng.dma_start(out=wmem_sb[dt], in_=r32(wmemT_d[dt * 128:(dt + 1) * 128, :]))
            ones_sb = p_const.tile([128, 1], F32R)
            nc.sync.dma_start(out=ones_sb, in_=r32(ones_d[:, :]))

            # ---- per-batch resident tiles (reused across batches) ----
            hmem_sb = p_batch.tile([128, nh, L], F32R)
            memnat_sb = p_batch.tile([128, nm, D], F32R)
            mbias_sb = p_batch.tile([128, nm], F32)

            for b in range(NB):
                # ---- stage A: h_memT = relu(W_memT.T @ memoryT) ----
                stage_a_anchor = {}
                for mblk in range(L // 512):
                    mT = [p_memT.tile([128, 512], F32R, tag=f"mT{dt}",
                                      name=f"mT{dt}") for dt in range(nd)]
                    for dt in range(nd):
                        eng = nc.gpsimd if (b == 0 and mblk == 0 and dt % 2) else nc.sync
                        eng.dma_start(
                            out=mT[dt],
                            in_=r32(memT_d[b, dt * 128:(dt + 1) * 128,
                                           mblk * 512:(mblk + 1) * 512]))
                    for ht in range(nh):
                        ps = p_mm.tile([128, 512], F32, tag="mm")
                        for dt in range(nd):
                            nc.tensor.matmul(
                                ps, wmem_sb[dt][:, ht * 128:(ht + 1) * 128],
                                mT[dt],
                                start=(dt == 0), stop=(dt == nd - 1))
                        rel_i = nc.scalar.activation(
                            hmem_sb[:, ht, mblk * 512:(mblk + 1) * 512], ps, AF.Relu)
                        if ht == nh - 1:
                            stage_a_anchor[mblk] = rel_i

                # deferred loads: needed from the first i-block onward, but
                # emitted after stage A's DMAs so they don't delay PE start
                if b == 0:
                    for dt in range(nd):
                        nc.sync.dma_start(out=win_sb[dt], in_=r32(winT_d[dt * 128:(dt + 1) * 128, :]))
                nc.sync.dma_start(out=mbias_sb, in_=mbias_d[b])

                # phase 1 of i-block ib: load inputsT block + h_inT.
                # Emitted one i-block AHEAD (software pipeline) so these PE
                # matmuls cover the softmax-normalize chain latency that the
                # gate phase of the previous i-block depends on.
                def emit_hin(ib):
                    isl = slice(ib * IBLK, (ib + 1) * IBLK)
                    inb = [p_inT.tile([128, IBLK], F32R, tag=f"inb{dt}",
                                      name=f"inb{dt}") for dt in range(nd)]
                    for dt in range(nd):
                        nc.sync.dma_start(
                            out=inb[dt],
                            in_=r32(inT_d[b, dt * 128:(dt + 1) * 128, isl]))
                    hin = p_hin.tile([128, nh, IBLK], F32R, name="hin")
                    for ht in range(nh):
                        ps = p_mm.tile([128, IBLK], F32, tag="mm", name="hin_ps")
                        for dt in range(nd):
                            nc.tensor.matmul(
                                ps, win_sb[dt][:, ht * 128:(ht + 1) * 128],
                                inb[dt],
                                start=(dt == 0), stop=(dt == nd - 1))
                        nc.scalar.activation(hin[:, ht, :], ps, AF.Relu)
                    return inb, hin

                cur = emit_hin(0)

                # Heavy deferred loads (first needed mid-first-i-block).
                # Gate their descriptor enqueue behind stage A's first relu
                # so they don't steal HBM bandwidth from the tiles the PE
                # needs to get started (they'd otherwise issue at t=0 on
                # parallel DGE queues).
                anchor = stage_a_anchor[0]
                for mt in range(nm):
                    dma_i = nc.sync.dma_start(
                        out=memnat_sb[:, mt, :],
                        in_=r32(mem_d[b, mt * 128:(mt + 1) * 128, :]))
                    if mt == 0:
                        bass_rust.add_dep_helper(
                            dma_i.ins, anchor.ins, sync=True,
                            reason="defer heavy prefetch past PE start")
                if b == 0:
                    for rt in range(ns):
                        nc.sync.dma_start(out=wres_sb[:, rt, :], in_=r32(wresT_d[rt * 128:(rt + 1) * 128, :]))

                # ---- i-block pipeline ----
                for ib in range(nib):
                    isl = slice(ib * IBLK, (ib + 1) * IBLK)
                    inb, hin = cur

                    # phase 2+3 (skewed): scores -> exp -> attended; the
                    # softmax denominator accumulates on the DVE (not PE)
                    att_ps = [p_att.tile([128, IBLK], F32, tag=f"att{dt}",
                                         name=f"att_ps{dt}")
                              for dt in range(nd)]
                    den_ps = p_att.tile([1, IBLK], F32, tag="den")
                    den_acc = p_sm.tile([128, IBLK], F32R, tag="den_acc")
                    sc_ps = [None] * nm
                    e_t = [None] * nm

                    def emit_scores(mt):
                        ps = p_mm.tile([128, IBLK], F32, tag="mm")
                        for ht in range(nh):
                            nc.tensor.matmul(
                                ps, hmem_sb[:, ht, mt * 128:(mt + 1) * 128],
                                hin[:, ht, :],
                                start=(ht == 0), stop=(ht == nh - 1))
                        sc_ps[mt] = ps

                    def emit_exp(mt):
                        e = p_E.tile([128, IBLK], F32R, tag="E")
                        nc.scalar.activation(
                            e, sc_ps[mt], AF.Exp,
                            bias=mbias_sb[:, mt:mt + 1], scale=scale)
                        e_t[mt] = e

                    def emit_att(mt):
                        e = e_t[mt]
                        for dt in range(nd):
                            nc.tensor.matmul(
                                att_ps[dt],
                                memnat_sb[:, mt, dt * 128:(dt + 1) * 128], e,
                                start=(mt == 0), stop=(mt == nm - 1))
                        # partial denominator on DVE: den_acc[p,i] += E[mt][p,i]
                        if mt == 0:
                            nc.vector.tensor_copy(den_acc, e.bitcast(F32))
                        else:
                            nc.vector.tensor_add(den_acc, den_acc, e.bitcast(F32))

                    emit_scores(0)
                    for mt in range(nm):
                        if mt + 1 < nm:
                            emit_scores(mt + 1)
                        emit_exp(mt)
                        emit_att(mt)

                    # single partition-sum matmul: denom[1,i] = ones.T @ den_acc
                    nc.tensor.matmul(den_ps, ones_sb, den_acc,
                                     start=True, stop=True)

                    # phase 4: normalize attT by softmax denominator
                    recip = p_sm.tile([1, IBLK], F32, tag="recip")
                    nc.vector.reciprocal(recip, den_ps)
                    bcast = p_sm.tile([128, IBLK], F32, tag="bc")
                    nc.gpsimd.partition_broadcast(bcast, recip)
                    attn = [p_attn.tile([128, IBLK], F32R, tag=f"attn{dt}",
                                        name=f"attn{dt}") for dt in range(nd)]
                    for dt in range(nd):
                        nc.vector.tensor_mul(attn[dt], att_ps[dt], bcast)

                    # pipeline: next i-block's inputsT load + h_inT matmuls go
                    # here in PE program order, covering the normalize chain
                    if ib + 1 < nib:
                        cur = emit_hin(ib + 1)

                    # phase 5: gate + output.  resT r-tile rt = inputsT (rt<nd)
                    # else attn.  out = resT * sigmoid(gateT).
                    # The first 3 s-tiles run their inputs-half (rt < nd)
                    # accumulation before any attn-dependent matmul, giving
                    # the PE ~2.6us of work that covers the normalize chain.
                    def res_tile(rt):
                        return inb[rt] if rt < nd else attn[rt - nd]

                    def gate_mms(ps, st, rts):
                        for rt in rts:
                            nc.tensor.matmul(
                                ps, wres_sb[:, rt, st * 128:(st + 1) * 128],
                                res_tile(rt),
                                start=(rt == 0), stop=(rt == ns - 1))

                    def gate_post(ps, st):
                        t = p_sm.tile([128, IBLK], F32, tag="t", name="t")
                        # sigmoid(x) = 0.5 + 0.5*tanh(x/2); tanh lives in the
                        # same ACT table set as exp -> no table reloads
                        nc.scalar.activation(t, ps, AF.Tanh, scale=0.5)
                        nc.vector.tensor_scalar(t, t, 0.5, 0.5, ALU.mult, ALU.add)
                        o = p_out.tile([128, IBLK], F32, tag="o", name="o")
                        nc.vector.tensor_mul(o, t, res_tile(st).bitcast(F32))
                        nc.sync.dma_start(
                            out=outT_d[b, st * 128:(st + 1) * 128, isl], in_=o)

                    npre = min(3, ns)
                    gate_ps = {}
                    for st in range(npre):
                        gate_ps[st] = p_mm.tile([128, IBLK], F32, tag="mm",
                                                name="gate_ps")
                        gate_mms(gate_ps[st], st, range(nd))
                    for st in range(ns):
                        if st < npre:
                            gate_mms(gate_ps[st], st, range(nd, ns))
                        else:
                            gate_ps[st] = p_mm.tile([128, IBLK], F32, tag="mm",
                                                    name="gate_ps")
                            gate_mms(gate_ps[st], st, range(ns))
                        gate_post(gate_ps[st], st)

    nc.compile()
    return nc


_PROGRAM_CACHE = {}


def _get_program(NB, L, D, H):
    key = (NB, L, D, H)
    if key not in _PROGRAM_CACHE:
        _PROGRAM_CACHE[key] = _build_program(NB, L, D, H)
    return _PROGRAM_CACHE[key]


def run(inputs, memory, mask, W_in, W_mem, W_res, trace=False):
    """Run the kernel; returns (output, BassKernelResults)."""
    B, L, D = inputs.shape
    H = W_in.shape[0]
    NB = B // N_CORES
    nm = L // 128

    nc = _get_program(NB, L, D, H)

    # host-side prep (all free): transposes + mask bias
    inputsT = np.ascontiguousarray(inputs.transpose(0, 2, 1))      # [B, D, L]
    memoryT = np.ascontiguousarray(memory.transpose(0, 2, 1))      # [B, D, L]
    memory_c = np.ascontiguousarray(memory)                        # [B, L, D]
    winT = np.ascontiguousarray(W_in.T)                            # [D, H]
    wmemT = np.ascontiguousarray(W_mem.T)                          # [D, H]
    wresT = np.ascontiguousarray(W_res.T)                          # [R, R]
    # bias per (b, m): 0 if unmasked else NEG_BIAS, laid out [B, 128, nm]
    # so that partition p, column j  <->  m = j*128 + p
    mb = np.where(mask, 0.0, NEG_BIAS).astype(np.float32)          # [B, L]
    mb = np.ascontiguousarray(mb.reshape(B, nm, 128).transpose(0, 2, 1))

    in_maps = []
    for c in range(N_CORES):
        bs = slice(c * NB, (c + 1) * NB)
        in_maps.append({
            "inT": inputsT[bs],
            "memT": memoryT[bs],
            "mem": memory_c[bs],
            "winT": winT,
            "wmemT": wmemT,
            "wresT": wresT,
            "mbias": mb[bs],
            "ones": np.ones((128, 1), np.float32),
        })

    res = run_bass_kernel_spmd(nc, in_maps, list(range(N_CORES)), trace=trace)

    # gather + un-transpose: outT [NB, R, L] per core -> [B, L, R]
    outs = [res.results[c]["outT"] for c in range(N_CORES)]
    outT = np.concatenate(outs, axis=0)                            # [B, R, L]
    out = np.ascontiguousarray(outT.transpose(0, 2, 1))            # [B, L, R]
    return out, res


def kernel(inputs, memory, mask, W_in, W_mem, W_res):
    out, _ = run(inputs, memory, mask, W_in, W_mem, W_res, trace=False)
    return out


# revision 28
# speedup vs baseline: 1.0195x; 1.0195x over previous
"""Trainium2 Bass kernel for DotAttention (nn_DotAttention_67963562492218).

Reference computation (per batch b):
    h_in  = relu(inputs @ W_in.T)            [Li, H]
    h_mem = relu(memory @ W_mem.T)           [Lm, H]
    S     = h_in @ h_mem.T / sqrt(H)         [Li, Lm]
    P     = softmax(where(mask, S, -inf))    [Li, Lm]
    att   = P @ memory                       [Li, D]
    res   = [inputs | att]                   [Li, 2D]
    out   = res * sigmoid(res @ W_res.T)     [Li, 2D]

Device strategy (8 cores, data-parallel over batch, 2 batch items/core):
  Everything on device lives in transposed ("feature-major") layout so that
  every matmul contracts over the partition axis with no on-device
  transposes.  Host pre-transposes inputs/memory/weights (free), and the
  output comes back transposed [2D, Li] and is un-transposed on host.

  - h_inT [h, i]  = W_inT.T @ inputsT     (lhsT = W_inT tile, rhs = inputsT)
  - h_memT[h, m]  = W_memT.T @ memoryT
  - S^T   [m, i]  = h_memT.T @ h_inT      -> exp(S/sqrt(H) + mask_bias[m])
      (softmax needs no max subtraction: scores are ~N(3.6, 0.5); masked
       entries get bias -1e4 so exp underflows to exactly 0)
  - attT  [d, i]  = mem_nat.T @ E         (mem natural tile is the lhsT!)
    denom [1, i]  = ones.T   @ E          (softmax denominator via matmul)
  - attT /= denom (broadcast via SBUF->SBUF DMA of the reciprocal row)
  - gateT [s, i]  = W_resT.T @ resT, resT = [inputsT ; attT] on partitions
  - outT = resT * (0.5 + 0.5*tanh(gateT/2))   (sigmoid via tanh: keeps the
    ACT engine on the exp_and_others table set the whole kernel)

All matmuls run as float32r (fp32 operands truncated to fp22 in the PE)
which streams at 1 cycle/row for free dim >= 256 -- bf16-class throughput
at ~2^-14 relative precision.
"""

import math
import numpy as np
from contextlib import ExitStack

import bass_rust
import concourse.bass as bass
import concourse.tile as tile
from concourse import bacc, mybir
from concourse.bass_utils import run_bass_kernel_spmd

F32 = mybir.dt.float32
F32R = mybir.dt.float32r
AF = mybir.ActivationFunctionType
ALU = mybir.AluOpType

N_CORES = 8
NEG_BIAS = -10000.0

# Full problem dims
FULL_B, FULL_L, FULL_D, FULL_H = 16, 2048, 512, 512


def r32(ap):
    return ap.bitcast(F32R)


def _build_program(NB, L, D, H, IBLK=512):
    """Build + compile the per-core Bass program.

    NB: batches per core; L: sequence length (Li == Lm); D: feature dim
    (Din == Dmem); H: hidden dim; R = 2*D (residual width).
    """
    R = 2 * D
    nd = D // 128   # d-tiles (contraction tiles for h_{in,mem}; partition tiles of attT)
    nh = H // 128   # h-tiles
    nm = L // 128   # m-tiles
    ns = R // 128   # s-tiles (= r-tiles)
    nib = L // IBLK  # i-blocks
    scale = 1.0 / math.sqrt(H)

    nc = bacc.Bacc("TRN2", target_bir_lowering=False)

    inT_d = nc.declare_dram_parameter("inT", [NB, D, L], F32, isOutput=False)
    memT_d = nc.declare_dram_parameter("memT", [NB, D, L], F32, isOutput=False)
    mem_d = nc.declare_dram_parameter("mem", [NB, L, D], F32, isOutput=False)
    winT_d = nc.declare_dram_parameter("winT", [D, H], F32, isOutput=False)
    wmemT_d = nc.declare_dram_parameter("wmemT", [D, H], F32, isOutput=False)
    wresT_d = nc.declare_dram_parameter("wresT", [R, R], F32, isOutput=False)
    mbias_d = nc.declare_dram_parameter("mbias", [NB, 128, nm], F32, isOutput=False)
    ones_d = nc.declare_dram_parameter("ones", [128, 1], F32, isOutput=False)
    outT_d = nc.declare_dram_parameter("outT", [NB, R, L], F32, isOutput=True)

    with tile.TileContext(nc) as tc:
        with ExitStack() as ctx:
            p_const = ctx.enter_context(tc.tile_pool(name="const", bufs=1))
            p_batch = ctx.enter_context(tc.tile_pool(name="batch", bufs=1))
            p_memT = ctx.enter_context(tc.tile_pool(name="memT", bufs=2))
            p_inT = ctx.enter_context(tc.tile_pool(name="inT", bufs=2))
            p_hin = ctx.enter_context(tc.tile_pool(name="hin", bufs=1))
            p_E = ctx.enter_context(tc.tile_pool(name="E", bufs=3))
            p_attn = ctx.enter_context(tc.tile_pool(name="attn", bufs=1))
            p_sm = ctx.enter_context(tc.tile_pool(name="sm", bufs=2))
            p_out = ctx.enter_context(tc.tile_pool(name="out", bufs=3))
            p_mm = ctx.enter_context(tc.tile_pool(name="mm", bufs=3, space="PSUM"))
            p_att = ctx.enter_context(tc.tile_pool(name="att", bufs=1, space="PSUM"))

            # ---- constants ----
            # Emission order here is DMA-queue order: load only what the PE
            # needs first (W_memT + first memoryT chunk), defer the rest so
            # the PE isn't stalled ~40us behind a serial DMA prologue.
            wres_sb = p_const.tile([128, ns, R], F32R)
            win_sb = [p_const.tile([128, H], F32R, name=f"win{dt}")
                      for dt in range(nd)]
            wmem_sb = [p_const.tile([128, H], F32R, name=f"wmem{dt}")
                       for dt in range(nd)]
            for dt in range(nd):
                # split the very first loads across HWDGE (sync) and SWDGE
                # (gpsimd) so they don't serialize on one DMA queue
                eng = nc.sync if dt % 2 == 0 else nc.gpsimd
                eng.dma_start(out=wmem_sb[dt], in_=r32(wmemT_d[dt * 128:(dt + 1) * 128, :]))
            ones_sb = p_const.tile([128, 1], F32R)
            nc.sync.dma_start(out=ones_sb, in_=r32(ones_d[:, :]))

            # ---- per-batch resident tiles (reused across batches) ----
            hmem_sb = p_batch.tile([128, nh, L], F32R)
            memnat_sb = p_batch.tile([128, nm, D], F32R)
            mbias_sb = p_batch.tile([128, nm], F32)

            # ---- stage A: h_memT = relu(W_memT.T @ memoryT) ----
            def emit_stage_a(b):
                anchor = None
                for mblk in range(L // 512):
                    mT = [p_memT.tile([128, 512], F32R, tag=f"mT{dt}",
                                      name=f"mT{dt}") for dt in range(nd)]
                    for dt in range(nd):
                        nc.sync.dma_start(
                            out=mT[dt],
                            in_=r32(memT_d[b, dt * 128:(dt + 1) * 128,
                                           mblk * 512:(mblk + 1) * 512]))
                    for ht in range(nh):
                        ps = p_mm.tile([128, 512], F32, tag="mm", name="hm_ps")
                        for dt in range(nd):
                            nc.tensor.matmul(
                                ps, wmem_sb[dt][:, ht * 128:(ht + 1) * 128],
                                mT[dt],
                                start=(dt == 0), stop=(dt == nd - 1))
                        rel_i = nc.scalar.activation(
                            hmem_sb[:, ht, mblk * 512:(mblk + 1) * 512], ps, AF.Relu)
                        if mblk == 0 and ht == nh - 1:
                            anchor = rel_i
                return anchor

            # Heavy deferred loads (first needed mid-first-i-block of the
            # batch).  Their descriptor enqueue is gated behind stage A's
            # first relu so they don't steal HBM bandwidth from the tiles
            # the PE needs to get started (all data DMA rides one HWDGE
            # queue, so enqueue order is bandwidth allocation).
            def emit_deferred(b, anchor):
                nc.sync.dma_start(out=mbias_sb, in_=mbias_d[b])
                for mt in range(nm):
                    dma_i = nc.sync.dma_start(
                        out=memnat_sb[:, mt, :],
                        in_=r32(mem_d[b, mt * 128:(mt + 1) * 128, :]))
                    if mt == 0:
                        bass_rust.add_dep_helper(
                            dma_i.ins, anchor.ins, sync=True,
                            reason="defer heavy prefetch past PE start")

            # phase 1 of i-block ib: load inputsT block + h_inT.
            # Emitted one i-block AHEAD (software pipeline) so these PE
            # matmuls cover the softmax-normalize chain latency that the
            # gate phase of the previous i-block depends on.
            def emit_hin(b, ib):
                isl = slice(ib * IBLK, (ib + 1) * IBLK)
                inb = [p_inT.tile([128, IBLK], F32R, tag=f"inb{dt}",
                                  name=f"inb{dt}") for dt in range(nd)]
                for dt in range(nd):
                    nc.sync.dma_start(
                        out=inb[dt],
                        in_=r32(inT_d[b, dt * 128:(dt + 1) * 128, isl]))
                hin = p_hin.tile([128, nh, IBLK], F32R, name="hin")
                for ht in range(nh):
                    ps = p_mm.tile([128, IBLK], F32, tag="mm", name="hin_ps")
                    for dt in range(nd):
                        nc.tensor.matmul(
                            ps, win_sb[dt][:, ht * 128:(ht + 1) * 128],
                            inb[dt],
                            start=(dt == 0), stop=(dt == nd - 1))
                    nc.scalar.activation(hin[:, ht, :], ps, AF.Relu)
                return inb, hin

            # ---- batch-0 prologue ----
            anchor0 = emit_stage_a(0)
            for dt in range(nd):
                nc.sync.dma_start(out=win_sb[dt], in_=r32(winT_d[dt * 128:(dt + 1) * 128, :]))
            cur = emit_hin(0, 0)
            emit_deferred(0, anchor0)
            for rt in range(ns):
                nc.sync.dma_start(out=wres_sb[:, rt, :], in_=r32(wresT_d[rt * 128:(rt + 1) * 128, :]))

            for b in range(NB):
                # ---- i-block pipeline ----
                for ib in range(nib):
                    isl = slice(ib * IBLK, (ib + 1) * IBLK)
                    inb, hin = cur

                    # phase 2+3 (skewed): scores -> exp -> attended; the
                    # softmax denominator accumulates on the DVE (not PE)
                    att_ps = [p_att.tile([128, IBLK], F32, tag=f"att{dt}",
                                         name=f"att_ps{dt}")
                              for dt in range(nd)]
                    den_ps = p_att.tile([1, IBLK], F32, tag="den")
                    den_acc = p_sm.tile([128, IBLK], F32R, tag="den_acc")
                    sc_ps = [None] * nm
                    e_t = [None] * nm

                    def emit_scores(mt):
                        ps = p_mm.tile([128, IBLK], F32, tag="mm")
                        for ht in range(nh):
                            nc.tensor.matmul(
                                ps, hmem_sb[:, ht, mt * 128:(mt + 1) * 128],
                                hin[:, ht, :],
                                start=(ht == 0), stop=(ht == nh - 1))
                        sc_ps[mt] = ps

                    def emit_exp(mt):
                        e = p_E.tile([128, IBLK], F32R, tag="E")
                        nc.scalar.activation(
                            e, sc_ps[mt], AF.Exp,
                            bias=mbias_sb[:, mt:mt + 1], scale=scale)
                        e_t[mt] = e

                    def emit_att(mt):
                        e = e_t[mt]
                        for dt in range(nd):
                            nc.tensor.matmul(
                                att_ps[dt],
                                memnat_sb[:, mt, dt * 128:(dt + 1) * 128], e,
                                start=(mt == 0), stop=(mt == nm - 1))
                        # partial denominator on DVE: den_acc[p,i] += E[mt][p,i]
                        if mt == 0:
                            nc.vector.tensor_copy(den_acc, e.bitcast(F32))
                        else:
                            nc.vector.tensor_add(den_acc, den_acc, e.bitcast(F32))

                    emit_scores(0)
                    for mt in range(nm):
                        if mt + 1 < nm:
                            emit_scores(mt + 1)
                        emit_exp(mt)
                        emit_att(mt)

                    # single partition-sum matmul: denom[1,i] = ones.T @ den_acc
                    nc.tensor.matmul(den_ps, ones_sb, den_acc,
                                     start=True, stop=True)

                    # phase 4: normalize attT by softmax denominator
                    recip = p_sm.tile([1, IBLK], F32, tag="recip")
                    nc.vector.reciprocal(recip, den_ps)
                    bcast = p_sm.tile([128, IBLK], F32, tag="bc")
                    nc.gpsimd.partition_broadcast(bcast, recip)
                    attn = [p_attn.tile([128, IBLK], F32R, tag=f"attn{dt}",
                                        name=f"attn{dt}") for dt in range(nd)]
                    for dt in range(nd):
                        nc.vector.tensor_mul(attn[dt], att_ps[dt], bcast)

                    # pipeline: the next work unit's PE matmuls go here in PE
                    # program order, covering the normalize chain.  At the
                    # end of a batch that unit is the NEXT batch's stage A +
                    # first h_inT.
                    if ib + 1 < nib:
                        cur = emit_hin(b, ib + 1)
                    elif b + 1 < NB:
                        anchor_n = emit_stage_a(b + 1)
                        emit_deferred(b + 1, anchor_n)
                        cur = emit_hin(b + 1, 0)

                    # phase 5: gate + output.  resT r-tile rt = inputsT (rt<nd)
                    # else attn.  out = resT * sigmoid(gateT).
                    # The first 3 s-tiles run their inputs-half (rt < nd)
                    # accumulation before any attn-dependent matmul, giving
                    # the PE ~2.6us of work that covers the normalize chain.
                    def res_tile(rt):
                        return inb[rt] if rt < nd else attn[rt - nd]

                    def gate_mms(ps, st, rts):
                        for rt in rts:
                            nc.tensor.matmul(
                                ps, wres_sb[:, rt, st * 128:(st + 1) * 128],
                                res_tile(rt),
                                start=(rt == 0), stop=(rt == ns - 1))

                    def gate_post(ps, st):
                        t = p_sm.tile([128, IBLK], F32, tag="t", name="t")
                        # sigmoid(x) = 0.5 + 0.5*tanh(x/2); tanh lives in the
                        # same ACT table set as exp -> no table reloads
                        nc.scalar.activation(t, ps, AF.Tanh, scale=0.5)
                        nc.vector.tensor_scalar(t, t, 0.5, 0.5, ALU.mult, ALU.add)
                        o = p_out.tile([128, IBLK], F32, tag="o", name="o")
                        nc.vector.tensor_mul(o, t, res_tile(st).bitcast(F32))
                        nc.sync.dma_start(
                            out=outT_d[b, st * 128:(st + 1) * 128, isl], in_=o)

                    npre = min(3, ns)
                    gate_ps = {}
                    for st in range(npre):
                        gate_ps[st] = p_mm.tile([128, IBLK], F32, tag="mm",
                                                name="gate_ps")
                        gate_mms(gate_ps[st], st, range(nd))
                    for st in range(ns):
                        if st < npre:
                            gate_mms(gate_ps[st], st, range(nd, ns))
                        else:
                            gate_ps[st] = p_mm.tile([128, IBLK], F32, tag="mm",
                                                    name="gate_ps")
                            gate_mms(gate_ps[st], st, range(ns))
                        gate_post(gate_ps[st], st)

    nc.compile()
    return nc


_PROGRAM_CACHE = {}


def _get_program(NB, L, D, H):
    key = (NB, L, D, H)
    if key not in _PROGRAM_CACHE:
        _PROGRAM_CACHE[key] = _build_program(NB, L, D, H)
    return _PROGRAM_CACHE[key]


def run(inputs, memory, mask, W_in, W_mem, W_res, trace=False):
    """Run the kernel; returns (output, BassKernelResults)."""
    B, L, D = inputs.shape
    H = W_in.shape[0]
    NB = B // N_CORES
    nm = L // 128

    nc = _get_program(NB, L, D, H)

    # host-side prep (all free): transposes + mask bias
    inputsT = np.ascontiguousarray(inputs.transpose(0, 2, 1))      # [B, D, L]
    memoryT = np.ascontiguousarray(memory.transpose(0, 2, 1))      # [B, D, L]
    memory_c = np.ascontiguousarray(memory)                        # [B, L, D]
    winT = np.ascontiguousarray(W_in.T)                            # [D, H]
    wmemT = np.ascontiguousarray(W_mem.T)                          # [D, H]
    wresT = np.ascontiguousarray(W_res.T)                          # [R, R]
    # bias per (b, m): 0 if unmasked else NEG_BIAS, laid out [B, 128, nm]
    # so that partition p, column j  <->  m = j*128 + p
    mb = np.where(mask, 0.0, NEG_BIAS).astype(np.float32)          # [B, L]
    mb = np.ascontiguousarray(mb.reshape(B, nm, 128).transpose(0, 2, 1))

    in_maps = []
    for c in range(N_CORES):
        bs = slice(c * NB, (c + 1) * NB)
        in_maps.append({
            "inT": inputsT[bs],
            "memT": memoryT[bs],
            "mem": memory_c[bs],
            "winT": winT,
            "wmemT": wmemT,
            "wresT": wresT,
            "mbias": mb[bs],
            "ones": np.ones((128, 1), np.float32),
        })

    res = run_bass_kernel_spmd(nc, in_maps, list(range(N_CORES)), trace=trace)

    # gather + un-transpose: outT [NB, R, L] per core -> [B, L, R]
    outs = [res.results[c]["outT"] for c in range(N_CORES)]
    outT = np.concatenate(outs, axis=0)                            # [B, R, L]
    out = np.ascontiguousarray(outT.transpose(0, 2, 1))            # [B, L, R]
    return out, res


def kernel(inputs, memory, mask, W_in, W_mem, W_res):
    out, _ = run(inputs, memory, mask, W_in, W_mem, W_res, trace=False)
    return out


# revision 32
# speedup vs baseline: 1.0234x; 1.0038x over previous
"""Trainium2 Bass kernel for DotAttention (nn_DotAttention_67963562492218).

Reference computation (per batch b):
    h_in  = relu(inputs @ W_in.T)            [Li, H]
    h_mem = relu(memory @ W_mem.T)           [Lm, H]
    S     = h_in @ h_mem.T / sqrt(H)         [Li, Lm]
    P     = softmax(where(mask, S, -inf))    [Li, Lm]
    att   = P @ memory                       [Li, D]
    res   = [inputs | att]                   [Li, 2D]
    out   = res * sigmoid(res @ W_res.T)     [Li, 2D]

Device strategy (8 cores, data-parallel over batch, 2 batch items/core):
  Everything on device lives in transposed ("feature-major") layout so that
  every matmul contracts over the partition axis with no on-device
  transposes.  Host pre-transposes inputs/memory/weights (free), and the
  output comes back transposed [2D, Li] and is un-transposed on host.

  - h_inT [h, i]  = W_inT.T @ inputsT     (lhsT = W_inT tile, rhs = inputsT)
  - h_memT[h, m]  = W_memT.T @ memoryT
  - S^T   [m, i]  = h_memT.T @ h_inT      -> exp(S/sqrt(H) + mask_bias[m])
      (softmax needs no max subtraction: scores are ~N(3.6, 0.5); masked
       entries get bias -1e4 so exp underflows to exactly 0)
  - attT  [d, i]  = mem_nat.T @ E         (mem natural tile is the lhsT!)
    denom [1, i]  = ones.T   @ E          (softmax denominator via matmul)
  - attT /= denom (broadcast via SBUF->SBUF DMA of the reciprocal row)
  - gateT [s, i]  = W_resT.T @ resT, resT = [inputsT ; attT] on partitions
  - outT = resT * (0.5 + 0.5*tanh(gateT/2))   (sigmoid via tanh: keeps the
    ACT engine on the exp_and_others table set the whole kernel)

All matmuls run as float32r (fp32 operands truncated to fp22 in the PE)
which streams at 1 cycle/row for free dim >= 256 -- bf16-class throughput
at ~2^-14 relative precision.
"""

import math
import numpy as np
from contextlib import ExitStack

import bass_rust
import concourse.bass as bass
import concourse.tile as tile
from concourse import bacc, mybir
from concourse.bass_utils import run_bass_kernel_spmd

F32 = mybir.dt.float32
F32R = mybir.dt.float32r
AF = mybir.ActivationFunctionType
ALU = mybir.AluOpType

N_CORES = 8
NEG_BIAS = -10000.0

# Full problem dims
FULL_B, FULL_L, FULL_D, FULL_H = 16, 2048, 512, 512


def r32(ap):
    return ap.bitcast(F32R)


def _build_program(NB, L, D, H, IBLK=512):
    """Build + compile the per-core Bass program.

    NB: batches per core; L: sequence length (Li == Lm); D: feature dim
    (Din == Dmem); H: hidden dim; R = 2*D (residual width).
    """
    R = 2 * D
    nd = D // 128   # d-tiles (contraction tiles for h_{in,mem}; partition tiles of attT)
    nh = H // 128   # h-tiles
    nm = L // 128   # m-tiles
    ns = R // 128   # s-tiles (= r-tiles)
    nib = L // IBLK  # i-blocks
    scale = 1.0 / math.sqrt(H)

    nc = bacc.Bacc("TRN2", target_bir_lowering=False)

    inT_d = nc.declare_dram_parameter("inT", [NB, D, L], F32, isOutput=False)
    memT_d = nc.declare_dram_parameter("memT", [NB, D, L], F32, isOutput=False)
    mem_d = nc.declare_dram_parameter("mem", [NB, L, D], F32, isOutput=False)
    winT_d = nc.declare_dram_parameter("winT", [D, H], F32, isOutput=False)
    wmemT_d = nc.declare_dram_parameter("wmemT", [D, H], F32, isOutput=False)
    wresT_d = nc.declare_dram_parameter("wresT", [R, R], F32, isOutput=False)
    mbias_d = nc.declare_dram_parameter("mbias", [NB, 128, nm], F32, isOutput=False)
    ones_d = nc.declare_dram_parameter("ones", [128, 1], F32, isOutput=False)
    outT_d = nc.declare_dram_parameter("outT", [NB, R, L], F32, isOutput=True)

    with tile.TileContext(nc) as tc:
        with ExitStack() as ctx:
            p_const = ctx.enter_context(tc.tile_pool(name="const", bufs=1))
            p_batch = ctx.enter_context(tc.tile_pool(name="batch", bufs=1))
            p_memT = ctx.enter_context(tc.tile_pool(name="memT", bufs=2))
            p_inT = ctx.enter_context(tc.tile_pool(name="inT", bufs=2))
            p_hin = ctx.enter_context(tc.tile_pool(name="hin", bufs=1))
            p_E = ctx.enter_context(tc.tile_pool(name="E", bufs=3))
            p_attn = ctx.enter_context(tc.tile_pool(name="attn", bufs=1))
            p_sm = ctx.enter_context(tc.tile_pool(name="sm", bufs=2))
            p_out = ctx.enter_context(tc.tile_pool(name="out", bufs=3))
            p_mm = ctx.enter_context(tc.tile_pool(name="mm", bufs=3, space="PSUM"))
            p_att = ctx.enter_context(tc.tile_pool(name="att", bufs=1, space="PSUM"))

            # ---- constants ----
            # Emission order here is DMA-queue order: load only what the PE
            # needs first (W_memT + first memoryT chunk), defer the rest so
            # the PE isn't stalled ~40us behind a serial DMA prologue.
            wres_sb = p_const.tile([128, ns, R], F32R)
            win_sb = [p_const.tile([128, H], F32R, name=f"win{dt}")
                      for dt in range(nd)]
            wmem_sb = [p_const.tile([128, H], F32R, name=f"wmem{dt}")
                       for dt in range(nd)]
            ones_sb = p_const.tile([128, 1], F32R)
            nc.sync.dma_start(out=ones_sb, in_=r32(ones_d[:, :]))

            # ---- per-batch resident tiles (reused across batches) ----
            hmem_sb = p_batch.tile([128, nh, L], F32R)
            memnat_sb = p_batch.tile([128, nm, D], F32R)
            mbias_sb = p_batch.tile([128, nm], F32)

            # ---- stage A: h_memT = relu(W_memT.T @ memoryT) ----
            # first=True (batch 0 only): the first memory block runs
            # dt-major with its DMAs interleaved per-dt so the very first
            # matmul needs only 0.5 MB of DMA (wmem[0] + mT[0]) instead of
            # 2 MB -- the PE starts ~8us earlier and HAM warms sooner.
            def emit_stage_a(b, first=False):
                anchor = None
                for mblk in range(L // 512):
                    mT = [p_memT.tile([128, 512], F32R, tag=f"mT{dt}",
                                      name=f"mT{dt}") for dt in range(nd)]
                    for dt in range(nd):
                        if first and mblk == 0:
                            nc.sync.dma_start(out=wmem_sb[dt], in_=r32(wmemT_d[dt * 128:(dt + 1) * 128, :]))
                        nc.sync.dma_start(
                            out=mT[dt],
                            in_=r32(memT_d[b, dt * 128:(dt + 1) * 128,
                                           mblk * 512:(mblk + 1) * 512]))
                    if first and mblk == 0:
                        # dt-major: 4 open PSUM groups (borrow the att tags,
                        # idle until the first i-block's attended phase)
                        pss = [p_att.tile([128, 512], F32, tag=f"att{ht}",
                                          name=f"hm0_ps{ht}") for ht in range(nh)]
                        for dt in range(nd):
                            for ht in range(nh):
                                nc.tensor.matmul(
                                    pss[ht], wmem_sb[dt][:, ht * 128:(ht + 1) * 128],
                                    mT[dt],
                                    start=(dt == 0), stop=(dt == nd - 1))
                        for ht in range(nh):
                            rel_i = nc.scalar.activation(
                                hmem_sb[:, ht, 0:512], pss[ht], AF.Relu)
                        anchor = rel_i
                        continue
                    for ht in range(nh):
                        ps = p_mm.tile([128, 512], F32, tag="mm", name="hm_ps")
                        for dt in range(nd):
                            nc.tensor.matmul(
                                ps, wmem_sb[dt][:, ht * 128:(ht + 1) * 128],
                                mT[dt],
                                start=(dt == 0), stop=(dt == nd - 1))
                        rel_i = nc.scalar.activation(
                            hmem_sb[:, ht, mblk * 512:(mblk + 1) * 512], ps, AF.Relu)
                        if mblk == 0 and ht == nh - 1:
                            anchor = rel_i
                return anchor

            # Heavy deferred loads (first needed mid-first-i-block of the
            # batch).  Their descriptor enqueue is gated behind stage A's
            # first relu so they don't steal HBM bandwidth from the tiles
            # the PE needs to get started (all data DMA rides one HWDGE
            # queue, so enqueue order is bandwidth allocation).
            def emit_deferred(b, anchor):
                nc.sync.dma_start(out=mbias_sb, in_=mbias_d[b])
                for mt in range(nm):
                    dma_i = nc.sync.dma_start(
                        out=memnat_sb[:, mt, :],
                        in_=r32(mem_d[b, mt * 128:(mt + 1) * 128, :]))
                    if mt == 0:
                        bass_rust.add_dep_helper(
                            dma_i.ins, anchor.ins, sync=True,
                            reason="defer heavy prefetch past PE start")

            # phase 1 of i-block ib: load inputsT block + h_inT.
            # Emitted one i-block AHEAD (software pipeline) so these PE
            # matmuls cover the softmax-normalize chain latency that the
            # gate phase of the previous i-block depends on.
            def emit_hin(b, ib):
                isl = slice(ib * IBLK, (ib + 1) * IBLK)
                inb = [p_inT.tile([128, IBLK], F32R, tag=f"inb{dt}",
                                  name=f"inb{dt}") for dt in range(nd)]
                for dt in range(nd):
                    nc.sync.dma_start(
                        out=inb[dt],
                        in_=r32(inT_d[b, dt * 128:(dt + 1) * 128, isl]))
                hin = p_hin.tile([128, nh, IBLK], F32R, name="hin")
                for ht in range(nh):
                    ps = p_mm.tile([128, IBLK], F32, tag="mm", name="hin_ps")
                    for dt in range(nd):
                        nc.tensor.matmul(
                            ps, win_sb[dt][:, ht * 128:(ht + 1) * 128],
                            inb[dt],
                            start=(dt == 0), stop=(dt == nd - 1))
                    nc.scalar.activation(hin[:, ht, :], ps, AF.Relu)
                return inb, hin

            # ---- batch-0 prologue ----
            anchor0 = emit_stage_a(0, first=True)
            for dt in range(nd):
                nc.sync.dma_start(out=win_sb[dt], in_=r32(winT_d[dt * 128:(dt + 1) * 128, :]))
            cur = emit_hin(0, 0)
            emit_deferred(0, anchor0)
            for rt in range(ns):
                nc.sync.dma_start(out=wres_sb[:, rt, :], in_=r32(wresT_d[rt * 128:(rt + 1) * 128, :]))

            for b in range(NB):
                # ---- i-block pipeline ----
                for ib in range(nib):
                    isl = slice(ib * IBLK, (ib + 1) * IBLK)
                    inb, hin = cur

                    # phase 2+3 (skewed): scores -> exp -> attended; the
                    # softmax denominator accumulates on the DVE (not PE)
                    att_ps = [p_att.tile([128, IBLK], F32, tag=f"att{dt}",
                                         name=f"att_ps{dt}")
                              for dt in range(nd)]
                    den_ps = p_att.tile([1, IBLK], F32, tag="den")
                    den_acc = p_sm.tile([128, IBLK], F32R, tag="den_acc")
                    sc_ps = [None] * nm
                    e_t = [None] * nm

                    def emit_scores(mt):
                        ps = p_mm.tile([128, IBLK], F32, tag="mm")
                        for ht in range(nh):
                            nc.tensor.matmul(
                                ps, hmem_sb[:, ht, mt * 128:(mt + 1) * 128],
                                hin[:, ht, :],
                                start=(ht == 0), stop=(ht == nh - 1))
                        sc_ps[mt] = ps

                    def emit_exp(mt):
                        e = p_E.tile([128, IBLK], F32R, tag="E")
                        nc.scalar.activation(
                            e, sc_ps[mt], AF.Exp,
                            bias=mbias_sb[:, mt:mt + 1], scale=scale)
                        e_t[mt] = e

                    def emit_att(mt):
                        e = e_t[mt]
                        for dt in range(nd):
                            nc.tensor.matmul(
                                att_ps[dt],
                                memnat_sb[:, mt, dt * 128:(dt + 1) * 128], e,
                                start=(mt == 0), stop=(mt == nm - 1))
                        # partial denominator on DVE: den_acc[p,i] += E[mt][p,i]
                        if mt == 0:
                            nc.vector.tensor_copy(den_acc, e.bitcast(F32))
                        else:
                            nc.vector.tensor_add(den_acc, den_acc, e.bitcast(F32))

                    emit_scores(0)
                    for mt in range(nm):
                        if mt + 1 < nm:
                            emit_scores(mt + 1)
                        emit_exp(mt)
                        emit_att(mt)

                    # single partition-sum matmul: denom[1,i] = ones.T @ den_acc
                    nc.tensor.matmul(den_ps, ones_sb, den_acc,
                                     start=True, stop=True)

                    # phase 4: normalize attT by softmax denominator
                    recip = p_sm.tile([1, IBLK], F32, tag="recip")
                    nc.vector.reciprocal(recip, den_ps)
                    bcast = p_sm.tile([128, IBLK], F32, tag="bc")
                    nc.gpsimd.partition_broadcast(bcast, recip)
                    attn = [p_attn.tile([128, IBLK], F32R, tag=f"attn{dt}",
                                        name=f"attn{dt}") for dt in range(nd)]
                    for dt in range(nd):
                        nc.vector.tensor_mul(attn[dt], att_ps[dt], bcast)

                    # pipeline: the next work unit's PE matmuls go here in PE
                    # program order, covering the normalize chain.  At the
                    # end of a batch that unit is the NEXT batch's stage A +
                    # first h_inT.
                    if ib + 1 < nib:
                        cur = emit_hin(b, ib + 1)
                    elif b + 1 < NB:
                        anchor_n = emit_stage_a(b + 1)
                        emit_deferred(b + 1, anchor_n)
                        cur = emit_hin(b + 1, 0)

                    # phase 5: gate + output.  resT r-tile rt = inputsT (rt<nd)
                    # else attn.  out = resT * sigmoid(gateT).
                    # The first 3 s-tiles run their inputs-half (rt < nd)
                    # accumulation before any attn-dependent matmul, giving
                    # the PE ~2.6us of work that covers the normalize chain.
                    def res_tile(rt):
                        return inb[rt] if rt < nd else attn[rt - nd]

                    def gate_mms(ps, st, rts):
                        for rt in rts:
                            nc.tensor.matmul(
                                ps, wres_sb[:, rt, st * 128:(st + 1) * 128],
                                res_tile(rt),
                                start=(rt == 0), stop=(rt == ns - 1))

                    def gate_post(ps, st):
                        t = p_sm.tile([128, IBLK], F32, tag="t", name="t")
                        # sigmoid(x) = 0.5 + 0.5*tanh(x/2); tanh lives in the
                        # same ACT table set as exp -> no table reloads
                        nc.scalar.activation(t, ps, AF.Tanh, scale=0.5)
                        nc.vector.tensor_scalar(t, t, 0.5, 0.5, ALU.mult, ALU.add)
                        o = p_out.tile([128, IBLK], F32, tag="o", name="o")
                        nc.vector.tensor_mul(o, t, res_tile(st).bitcast(F32))
                        nc.sync.dma_start(
                            out=outT_d[b, st * 128:(st + 1) * 128, isl], in_=o)

                    # 3 mm-pool slots + the denominator bank (free once the
                    # reciprocal has read it) give 4 early inputs-half chunks
                    npre = min(4, ns)
                    gate_ps = {}
                    for st in range(npre):
                        if st < 3:
                            gate_ps[st] = p_mm.tile([128, IBLK], F32, tag="mm",
                                                    name="gate_ps")
                        else:
                            gate_ps[st] = p_att.tile([128, IBLK], F32, tag="den",
                                                     name="gate_ps_den")
                        gate_mms(gate_ps[st], st, range(nd))
                    for st in range(ns):
                        if st < npre:
                            gate_mms(gate_ps[st], st, range(nd, ns))
                        else:
                            gate_ps[st] = p_mm.tile([128, IBLK], F32, tag="mm",
                                                    name="gate_ps")
                            gate_mms(gate_ps[st], st, range(ns))
                        gate_post(gate_ps[st], st)

    nc.compile()
    return nc


_PROGRAM_CACHE = {}


def _get_program(NB, L, D, H):
    key = (NB, L, D, H)
    if key not in _PROGRAM_CACHE:
        _PROGRAM_CACHE[key] = _build_program(NB, L, D, H)
    return _PROGRAM_CACHE[key]


def run(inputs, memory, mask, W_in, W_mem, W_res, trace=False):
    """Run the kernel; returns (output, BassKernelResults)."""
    B, L, D = inputs.shape
    H = W_in.shape[0]
    NB = B // N_CORES
    nm = L // 128

    nc = _get_program(NB, L, D, H)

    # host-side prep (all free): transposes + mask bias
    inputsT = np.ascontiguousarray(inputs.transpose(0, 2, 1))      # [B, D, L]
    memoryT = np.ascontiguousarray(memory.transpose(0, 2, 1))      # [B, D, L]
    memory_c = np.ascontiguousarray(memory)                        # [B, L, D]
    winT = np.ascontiguousarray(W_in.T)                            # [D, H]
    wmemT = np.ascontiguousarray(W_mem.T)                          # [D, H]
    wresT = np.ascontiguousarray(W_res.T)                          # [R, R]
    # bias per (b, m): 0 if unmasked else NEG_BIAS, laid out [B, 128, nm]
    # so that partition p, column j  <->  m = j*128 + p
    mb = np.where(mask, 0.0, NEG_BIAS).astype(np.float32)          # [B, L]
    mb = np.ascontiguousarray(mb.reshape(B, nm, 128).transpose(0, 2, 1))

    in_maps = []
    for c in range(N_CORES):
        bs = slice(c * NB, (c + 1) * NB)
        in_maps.append({
            "inT": inputsT[bs],
            "memT": memoryT[bs],
            "mem": memory_c[bs],
            "winT": winT,
            "wmemT": wmemT,
            "wresT": wresT,
            "mbias": mb[bs],
            "ones": np.ones((128, 1), np.float32),
        })

    res = run_bass_kernel_spmd(nc, in_maps, list(range(N_CORES)), trace=trace)

    # gather + un-transpose: outT [NB, R, L] per core -> [B, L, R]
    outs = [res.results[c]["outT"] for c in range(N_CORES)]
    outT = np.concatenate(outs, axis=0)                            # [B, R, L]
    out = np.ascontiguousarray(outT.transpose(0, 2, 1))            # [B, L, R]
    return out, res


def kernel(inputs, memory, mask, W_in, W_mem, W_res):
    out, _ = run(inputs, memory, mask, W_in, W_mem, W_res, trace=False)
    return out
